# revision 10
# baseline (speedup 1.0000x reference)
"""Trainium2 Bass kernel for nn_Attention (dense transformer attention block).

Full-input contract: kernel(**inputs) takes the unsharded inputs and returns
the full output. Internally: 8 NeuronCores, tensor-parallel over head groups
(4 heads each) x data-parallel over batch (2), core c = b*4 + g. Each core
computes q/k/v projections for its head group (columns of Wq/Wk/Wv), RoPE,
causal flash-style attention (transposed-P layout, no max subtraction), and a
partial o_proj with its rows of Wo. The 4 partial outputs per batch element
are summed on the host (the all-reduce of the row-sharded o_proj).

Matmul dtype is configurable (DTYPE): fp16 default (inputs cast on host,
~7e-4 rel err), float32r fallback (~3.5e-4).
"""
import contextlib
import numpy as np
import concourse.bass as bass
from concourse import bacc
import concourse.mybir as mybir
import concourse.tile as tile
from concourse.bass_utils import run_bass_kernel_spmd

F32 = mybir.dt.float32
F32R = mybir.dt.float32r
F16 = mybir.dt.float16
BF16 = mybir.dt.bfloat16
EXP = mybir.ActivationFunctionType.Exp
MMDT = {"f32r": F32R, "f16": F16, "bf16": BF16}

S = 2048          # sequence length
HID = 2048        # hidden size
D = 128           # head dim
GH = 4            # heads per core
GW = GH * D       # 512, projection width per core
NCORES = 8
SC = S // 512     # 4 column chunks of 512
HC = HID // 128   # 16 contraction chunks
SCALE = float(D) ** -0.5
NEG = -1.0e30

DTYPE = "f16"     # matmul dtype: 'f16' | 'bf16' | 'f32r'


def _build(variant, dt):
    """variant: 'causal' or 'full'; dt: matmul dtype key."""
    MDT = MMDT[dt]
    two_byte = dt in ("f16", "bf16")
    IDT = MDT if two_byte else F32          # dram dtype of matmul inputs
    nc = bacc.Bacc("TRN2", target_bir_lowering=False, debug=False,
                   num_devices=NCORES)
    xt = nc.dram_tensor("xt", [HID, S], IDT, kind="ExternalInput").ap()
    wq = nc.dram_tensor("wq", [HID, GW], IDT, kind="ExternalInput").ap()
    wk = nc.dram_tensor("wk", [HID, GW], IDT, kind="ExternalInput").ap()
    wv = nc.dram_tensor("wv", [HID, GW], IDT, kind="ExternalInput").ap()
    wo = nc.dram_tensor("wo", [GW, HID], IDT, kind="ExternalInput").ap()
    cost = nc.dram_tensor("cost", [D, S], F32, kind="ExternalInput").ap()
    sint = nc.dram_tensor("sint", [D, S], F32, kind="ExternalInput").ap()
    btpl = nc.dram_tensor("btpl", [D, 896], IDT, kind="ExternalInput").ap()
    out = nc.dram_tensor("out", [S, HID], F32, kind="ExternalOutput").ap()

    def _bc(ap):
        return ap if two_byte else ap.bitcast(F32R)

    xt_r = _bc(xt.rearrange("(c p) s -> p c s", p=128))   # [128, 16, 2048]
    wq_r = _bc(wq.rearrange("(c p) m -> p c m", p=128))   # [128, 16, 512]
    wk_r = _bc(wk.rearrange("(c p) m -> p c m", p=128))
    wv_r = _bc(wv.rearrange("(c p) m -> p c m", p=128))
    wo_r = _bc(wo.rearrange("(c p) m -> p c m", p=128))   # [128, 4, 2048]

    # xt DMA batching: 4 h-chunks per dma for fine-grained deps
    XB = 4
    NXT = HC // XB
    XBUFS = 2 * NXT if two_byte else NXT + 2

    with tile.TileContext(nc) as tc:
        with contextlib.ExitStack() as ctx:
            persist = ctx.enter_context(tc.tile_pool(name="persist", bufs=1))
            psum = ctx.enter_context(tc.tile_pool(name="psum", bufs=1, space="PSUM"))

            _n = [0]

            def bank(i, shape=(128, 512)):
                _n[0] += 1
                return psum.tile(list(shape), F32, tag=f"b{i}", name=f"bk{i}_{_n[0]}")

            qts = [[persist.tile([128, 512], MDT, tag=f"qt{h}_{s}",
                                 name=f"qt{h}_{s}") for s in range(SC)]
                   for h in range(GH)]
            kts = [[persist.tile([128, 512], MDT, tag=f"kt{h}_{s}",
                                 name=f"kt{h}_{s}") for s in range(SC)]
                   for h in range(GH)]
            vts = [persist.tile([128, GW], MDT, tag=f"v{st}", name=f"v{st}")
                   for st in range(HC)]
            cos_sb = persist.tile([128, S], F32, tag="cos")
            sin_sb = persist.tile([128, S], F32, tag="sin")
            btpl_sb = persist.tile([128, 896], MDT, tag="btpl")
            ones_f = persist.tile([128, 1], F32, tag="onesf")
            ones = persist.tile([128, 1], MDT, tag="ones")

            # ---- Phase A: projections ----------------------------------
            with tc.tile_pool(name="phA", bufs=1) as pha:
                if two_byte:
                    wv_cs = []
                    for j in range(NXT):
                        wvc = pha.tile([128, XB, GW], MDT, tag="wvf", bufs=NXT,
                                       name=f"wvf_{j}")
                        nc.sync.dma_start(out=wvc,
                                          in_=wv_r[:, j * XB:(j + 1) * XB, :])
                        wv_cs.append(wvc)

                def xt_tile(sc, j):
                    t = pha.tile([128, XB, 512], MDT, tag="xt", bufs=XBUFS,
                                 name=f"xt_{sc}_{j}")
                    nc.sync.dma_start(
                        out=t, in_=xt_r[:, j * XB:(j + 1) * XB,
                                        sc * 512:(sc + 1) * 512])
                    return t

                # --- A1: V = X @ Wv, natural layout [s, dv] -------------
                for sc in range(SC):
                    vps = [bank(4 + st) for st in range(4)]
                    for j in range(NXT):
                        xt_t = xt_tile(sc, j)
                        for hh in range(XB):
                            h = j * XB + hh
                            if two_byte:
                                wv_t = wv_cs[j][:, hh, :]
                            else:
                                wv_c = pha.tile([128, GW], MDT, tag="wc",
                                                bufs=3, name=f"wv_{sc}_{h}")
                                nc.sync.dma_start(out=wv_c, in_=wv_r[:, h, :])
                                wv_t = wv_c
                            for st in range(4):
                                nc.tensor.matmul(
                                    vps[st], xt_t[:, hh, st * 128:(st + 1) * 128],
                                    wv_t, start=(h == 0), stop=(h == HC - 1))
                    for st in range(4):
                        nc.scalar.copy(vts[sc * 4 + st], vps[st])

                # wq/wk resident, loaded during A1
                wq_sb = pha.tile([128, HC, GW], MDT, tag="wqk", bufs=2)
                wk_sb = pha.tile([128, HC, GW], MDT, tag="wqk", bufs=2)
                nc.sync.dma_start(out=wq_sb, in_=wq_r)
                nc.sync.dma_start(out=wk_sb, in_=wk_r)
                nc.sync.dma_start(out=cos_sb, in_=cost)
                nc.sync.dma_start(out=sin_sb, in_=sint)
                nc.sync.dma_start(out=btpl_sb, in_=_bc(btpl))
                nc.vector.memset(ones_f, 1.0)
                nc.vector.tensor_copy(ones, ones_f)

                # --- A2: Qt/Kt = W^T X^T with fused RoPE ----------------
                # PSUM banks are freed fast by a single ACT copy to SBUF;
                # RoPE runs on DVE off the critical path.
                for sc in range(SC):
                    ssl = slice(sc * 512, (sc + 1) * 512)
                    qps = [bank(d) for d in range(GH)]
                    kps = [bank(4 + d) for d in range(GH)]
                    for j in range(NXT):
                        xt_t = xt_tile(sc, j)
                        for hh in range(XB):
                            h = j * XB + hh
                            for d in range(GH):
                                nc.tensor.matmul(
                                    qps[d], wq_sb[:, h, d * 128:(d + 1) * 128],
                                    xt_t[:, hh, :], start=(h == 0),
                                    stop=(h == HC - 1))
                            for d in range(GH):
                                nc.tensor.matmul(
                                    kps[d], wk_sb[:, h, d * 128:(d + 1) * 128],
                                    xt_t[:, hh, :], start=(h == 0),
                                    stop=(h == HC - 1))
                    for d in range(GH):
                        for ps_t, dsl in ((qps[d], qts[d][sc]), (kps[d], kts[d][sc])):
                            t2 = pha.tile([128, 512], F32, tag="t2", bufs=8,
                                          name=f"t2_{sc}_{d}_{dsl.tensor.name}")
                            nc.scalar.copy(t2, ps_t)       # frees the bank
                            t1 = pha.tile([128, 512], F32, tag="t1", bufs=4,
                                          name=f"t1_{sc}_{d}_{dsl.tensor.name}")
                            nc.vector.tensor_mul(t1, t2, cos_sb[:, ssl])
                            nc.vector.tensor_mul(dsl[0:64, :], t2[64:128, :],
                                                 sin_sb[64:128, ssl])
                            nc.vector.tensor_mul(dsl[64:128, :], t2[0:64, :],
                                                 sin_sb[0:64, ssl])
                            nc.vector.tensor_add(dsl, dsl, t1)

            # ---- Phase B: attention + o_proj, per 512-wide q chunk -----
            with tc.tile_pool(name="phB", bufs=1) as phb:
                wo_sb = phb.tile([128, GH, HID], MDT, tag="wo")
                nc.sync.dma_start(out=wo_sb, in_=wo_r)

                for qc in range(SC):
                    qsl = slice(qc * 512, (qc + 1) * 512)
                    # diagonal blocks first so their DVE mask-add latency
                    # hides under subsequent off-diagonal PE work
                    if variant == "causal":
                        order = list(range(4 * qc, 4 * qc + 4)) + list(range(4 * qc))
                    else:
                        order = list(range(HC))
                    nkb = len(order)
                    ctx_t = []
                    for hd in range(GH):
                        ctxps = bank(2 + (hd % 2))
                        denps = bank(4 + (hd % 2), shape=(1, 512))
                        pend = []  # software pipeline: delay ctx/den by 1

                        def flush(last, ctxps=ctxps, denps=denps, hd=hd):
                            pexp, first, kbp = pend.pop()
                            nc.tensor.matmul(ctxps,
                                             vts[kbp][:, hd * 128:(hd + 1) * 128],
                                             pexp, start=first, stop=last)
                            nc.tensor.matmul(denps, ones, pexp,
                                             start=first, stop=last)

                        for i, kb in enumerate(order):
                            sps = bank(i % 2)
                            nc.tensor.matmul(
                                sps, kts[hd][kb // 4][:, (kb % 4) * 128:(kb % 4 + 1) * 128],
                                qts[hd][qc], start=True, stop=True)
                            pexp = phb.tile([128, 512], MDT, tag="pexp", bufs=4,
                                            name=f"pexp_{qc}_{hd}_{kb}")
                            nc.scalar.activation(pexp, sps, EXP, scale=SCALE)
                            if variant == "causal" and kb >= 4 * qc:
                                # zero the masked triangle: multiplicative
                                # 0/1 mask on gpsimd, only over the partially
                                # masked column range
                                j = kb - 4 * qc
                                off = (3 - j) * 128
                                w = (j + 1) * 128
                                nc.gpsimd.tensor_mul(pexp[:, 0:w], pexp[:, 0:w],
                                                     btpl_sb[:, off:off + w])
                            if pend:
                                flush(False)
                            pend.append((pexp, i == 0, kb))
                        flush(True)

                        # denom: ACT evict -> gpsimd broadcast -> DVE recip
                        dsm = phb.tile([1, 512], F32, tag="dsm", bufs=2,
                                       name=f"dsm_{qc}_{hd}")
                        nc.scalar.copy(dsm, denps)
                        dbc = phb.tile([128, 512], F32, tag="dbc", bufs=2,
                                       name=f"dbc_{qc}_{hd}")
                        nc.gpsimd.partition_broadcast(dbc, dsm)
                        dscr = phb.tile([128, 512], F32, tag="dscr", bufs=2,
                                        name=f"dscr_{qc}_{hd}")
                        drc = phb.tile([128, 512], F32, tag="drc", bufs=2,
                                       name=f"drc_{qc}_{hd}")
                        nc.vector.reciprocal_approx_accurate(out=drc, in_=dbc,
                                                             scratch=dscr)
                        ct = phb.tile([128, 512], MDT, tag="ctx", bufs=6,
                                      name=f"ctx_{qc}_{hd}")
                        nc.vector.tensor_mul(ct, ctxps, drc)
                        ctx_t.append(ct)

                    for qb in range(4):
                        orow = phb.tile([128, HID], F32, tag="orow", bufs=2,
                                        name=f"orow_{qc}_{qb}")
                        for ob in range(4):
                            ops = bank(6 + (ob % 2))
                            for hd in range(GH):
                                nc.tensor.matmul(
                                    ops, ctx_t[hd][:, qb * 128:(qb + 1) * 128],
                                    wo_sb[:, hd, ob * 512:(ob + 1) * 512],
                                    start=(hd == 0), stop=(hd == GH - 1))
                            nc.vector.tensor_copy(orow[:, ob * 512:(ob + 1) * 512], ops)
                        nc.sync.dma_start(
                            out=out[(qc * 4 + qb) * 128:(qc * 4 + qb + 1) * 128, :],
                            in_=orow)
    nc.compile()
    return nc


_CACHE = {}


def _get(variant, dt=None):
    dt = dt or DTYPE
    if (variant, dt) not in _CACHE:
        _CACHE[(variant, dt)] = _build(variant, dt)
    return _CACHE[(variant, dt)]


def _rope_tables():
    inv = 1.0 / (10000.0 ** (np.arange(0, D, 2, dtype=np.float64) / D))  # [64]
    t = np.arange(S, dtype=np.float64)
    fr = np.outer(inv, t)                       # [64, S]
    cosT = np.concatenate([np.cos(fr), np.cos(fr)], 0).astype(np.float32)  # [128, S]
    # partition-swapped sign-folded sin: rows 0:64 = +sin (applied to upper
    # half of rope), rows 64:128 = -sin (applied to lower half)
    sinT = np.concatenate([np.sin(fr), -np.sin(fr)], 0).astype(np.float32)
    return cosT, sinT


def _btpl_causal():
    # binary keep-mask: 0 where k > c-384 (masked), 1 elsewhere
    k = np.arange(128)[:, None]
    c = np.arange(896)[None, :]
    return np.where(k > c - 384, np.float32(0.0), np.float32(1.0)).astype(np.float32)


def _np_cast(a, dt):
    if dt == "f16":
        return a.astype(np.float16)
    if dt == "bf16":
        import ml_dtypes
        return a.astype(ml_dtypes.bfloat16)
    return a


def _numpy_fallback(hs, Wq, Wk, Wv, Wo, mask):
    B = hs.shape[0]
    cosT, sinT = _rope_tables()
    cos = cosT.T[None, :, None, :]              # [1, S, 1, 128]
    sin = np.abs(sinT).T[None, :, None, :]
    outs = []
    for b in range(B):
        x = hs[b]
        q = (x @ Wq).reshape(S, 16, D)[None]
        k = (x @ Wk).reshape(S, 16, D)[None]
        vv = (x @ Wv).reshape(S, 16, D)

        def rope(z):
            z1, z2 = z[..., :64], z[..., 64:]
            rot = np.concatenate([-z2, z1], -1)
            return z * cos + rot * sin

        q, k = rope(q)[0], rope(k)[0]
        o = np.empty((S, 16, D), np.float32)
        m = mask[0, 0]
        for h in range(16):
            sc = (q[:, h] @ k[:, h].T) * SCALE
            sc = np.where(m == 0, -np.inf, sc)
            sc -= sc.max(-1, keepdims=True)
            p = np.exp(sc)
            p /= p.sum(-1, keepdims=True)
            o[:, h] = p @ vv[:, h]
        outs.append(o.reshape(S, HID) @ Wo)
    return np.stack(outs).astype(np.float32)


def kernel(hidden_states, Wq, Wk, Wv, Wo, attention_mask):
    hs = np.asarray(hidden_states, dtype=np.float32)
    Wq, Wk, Wv, Wo = (np.asarray(w, dtype=np.float32) for w in (Wq, Wk, Wv, Wo))
    mask = np.asarray(attention_mask)
    B = hs.shape[0]

    m2 = mask.reshape(mask.shape[-2], mask.shape[-1])
    if np.all(m2 == 1):
        variant = "full"
    elif np.array_equal(m2 != 0, np.tril(np.ones((S, S), dtype=bool))):
        variant = "causal"
    else:
        return _numpy_fallback(hs, Wq, Wk, Wv, Wo, mask)

    cosT, sinT = _rope_tables()
    btpl = _btpl_causal() if variant == "causal" else np.ones((128, 896), np.float32)

    in_maps = []
    for c in range(NCORES):
        b, g = divmod(c, GH)
        gsl = slice(g * GW, (g + 1) * GW)
        in_maps.append({
            "xt": _np_cast(np.ascontiguousarray(hs[b].T), DTYPE),
            "wq": _np_cast(np.ascontiguousarray(Wq[:, gsl]), DTYPE),
            "wk": _np_cast(np.ascontiguousarray(Wk[:, gsl]), DTYPE),
            "wv": _np_cast(np.ascontiguousarray(Wv[:, gsl]), DTYPE),
            "wo": _np_cast(np.ascontiguousarray(Wo[gsl, :]), DTYPE),
            "cost": cosT, "sint": sinT, "btpl": _np_cast(btpl, DTYPE),
        })

    nc = _get(variant)
    res = run_bass_kernel_spmd(nc, in_maps, list(range(NCORES))).results
    out = np.zeros((B, S, HID), np.float32)
    for c in range(NCORES):
        b = c // GH
        out[b] += res[c]["out"]
    return out


# revision 14
# speedup vs baseline: 1.3551x; 1.3551x over previous
"""Trainium2 Bass kernel for nn_Attention (dense transformer attention block).

Full-input contract: kernel(**inputs) takes the unsharded inputs and returns
the full output. 8 NeuronCores: tensor-parallel over head groups (4 heads) x
data-parallel over batch (2); core c = b*4 + g. Per core: q/k/v projections
for its head group, RoPE, causal flash-style attention (transposed-P layout,
softmax without max-subtraction), partial o_proj with its rows of Wo; the 4
partials per batch element are summed on the host (the all-reduce of the
row-sharded o_proj).

The Q/K projection work (phase A2) is interleaved into the attention stream
(phase B) as PE filler: B-qc only needs Q/K columns of chunks <= qc, so
A2-sc(qc+1) runs alongside B-qc, hiding the exp/softmax latency chains.

Matmul dtype configurable (DTYPE): fp16 default (~7e-4 rel err), f32r
fallback (~3.5e-4).
"""
import contextlib
import numpy as np
import concourse.bass as bass
from concourse import bacc
import concourse.mybir as mybir
import concourse.tile as tile
from concourse.bass_utils import run_bass_kernel_spmd

F32 = mybir.dt.float32
F32R = mybir.dt.float32r
F16 = mybir.dt.float16
BF16 = mybir.dt.bfloat16
EXP = mybir.ActivationFunctionType.Exp
MMDT = {"f32r": F32R, "f16": F16, "bf16": BF16}

S = 2048
HID = 2048
D = 128
GH = 4            # heads per core
GW = GH * D       # 512
NCORES = 8
SC = S // 512     # 4 column chunks
HC = HID // 128   # 16 contraction chunks
SCALE = float(D) ** -0.5
NEG = -1.0e30

DTYPE = "f16"     # matmul dtype: 'f16' | 'bf16' | 'f32r'


def _build(variant, dt):
    MDT = MMDT[dt]
    two_byte = dt in ("f16", "bf16")
    IDT = MDT if two_byte else F32
    nc = bacc.Bacc("TRN2", target_bir_lowering=False, debug=False,
                   num_devices=NCORES)
    xt = nc.dram_tensor("xt", [HID, S], IDT, kind="ExternalInput").ap()
    wq = nc.dram_tensor("wq", [HID, GW], IDT, kind="ExternalInput").ap()
    wk = nc.dram_tensor("wk", [HID, GW], IDT, kind="ExternalInput").ap()
    wv = nc.dram_tensor("wv", [HID, GW], IDT, kind="ExternalInput").ap()
    wo = nc.dram_tensor("wo", [GW, HID], IDT, kind="ExternalInput").ap()
    cost = nc.dram_tensor("cost", [D, S], F32, kind="ExternalInput").ap()
    sint = nc.dram_tensor("sint", [D, S], F32, kind="ExternalInput").ap()
    btpl = nc.dram_tensor("btpl", [D, 896], F32, kind="ExternalInput").ap()
    out = nc.dram_tensor("out", [S, HID], F32, kind="ExternalOutput").ap()

    def _bc(ap):
        return ap if two_byte else ap.bitcast(F32R)

    xt_r = _bc(xt.rearrange("(c p) s -> p c s", p=128))   # [128, 16, 2048]
    wq_r = _bc(wq.rearrange("(c p) m -> p c m", p=128))   # [128, 16, 512]
    wk_r = _bc(wk.rearrange("(c p) m -> p c m", p=128))
    wv_r = _bc(wv.rearrange("(c p) m -> p c m", p=128))
    wo_r = _bc(wo.rearrange("(c p) m -> p c m", p=128))   # [128, 4, 2048]

    XB = 4                   # h-chunks per xt DMA
    NXT = HC // XB           # 4 xt tiles per s-chunk

    with tile.TileContext(nc) as tc:
        with contextlib.ExitStack() as ctx:
            persist = ctx.enter_context(tc.tile_pool(name="persist", bufs=1))
            psum = ctx.enter_context(tc.tile_pool(name="psum", bufs=1, space="PSUM"))
            work = ctx.enter_context(tc.tile_pool(name="work", bufs=1))

            _n = [0]

            def bank(i, shape=(128, 512)):
                _n[0] += 1
                return psum.tile(list(shape), F32, tag=f"b{i}", name=f"bk{i}_{_n[0]}")

            qts = [[persist.tile([128, 512], MDT, tag=f"qt{h}_{s}",
                                 name=f"qt{h}_{s}") for s in range(SC)]
                   for h in range(GH)]
            kts = [[persist.tile([128, 512], MDT, tag=f"kt{h}_{s}",
                                 name=f"kt{h}_{s}") for s in range(SC)]
                   for h in range(GH)]
            vts = [persist.tile([128, GW], MDT, tag=f"v{st}", name=f"v{st}")
                   for st in range(HC)]
            cos_sb = persist.tile([128, S], F32, tag="cos")
            sin_sb = persist.tile([128, S], F32, tag="sin")
            btpl_sb = persist.tile([128, 896], F32, tag="btpl")
            ones_f = persist.tile([128, 1], F32, tag="onesf")
            ones = persist.tile([128, 1], MDT, tag="ones")
            wo_sb = persist.tile([128, GH, HID], MDT, tag="wo")

            def xt_tile(sc, j):
                t = work.tile([128, XB, 512], MDT, tag="xt", bufs=6,
                              name=f"xt_{sc}_{j}")
                nc.sync.dma_start(
                    out=t, in_=xt_r[:, j * XB:(j + 1) * XB,
                                    sc * 512:(sc + 1) * 512])
                return t

            # ---- A1: V = X @ Wv (banks b0..b3) -------------------------
            with tc.tile_pool(name="phV", bufs=1) as phv:
                wv_cs = []
                for j in range(NXT):
                    wvc = phv.tile([128, XB, GW], MDT, tag="wvf", bufs=NXT,
                                   name=f"wvf_{j}")
                    nc.sync.dma_start(out=wvc, in_=wv_r[:, j * XB:(j + 1) * XB, :])
                    wv_cs.append(wvc)
                for sc in range(SC):
                    vps = [bank(st) for st in range(4)]
                    for j in range(NXT):
                        xt_t = xt_tile(sc, j)
                        for hh in range(XB):
                            h = j * XB + hh
                            for st in range(4):
                                nc.tensor.matmul(
                                    vps[st], xt_t[:, hh, st * 128:(st + 1) * 128],
                                    wv_cs[j][:, hh, :],
                                    start=(h == 0), stop=(h == HC - 1))
                    for st in range(4):
                        nc.scalar.copy(vts[sc * 4 + st], vps[st])

            # weights for Q/K/O + tables, loaded during A1
            wq_sb = work.tile([128, HC, GW], MDT, tag="wqk", bufs=2)
            wk_sb = work.tile([128, HC, GW], MDT, tag="wqk", bufs=2)
            nc.sync.dma_start(out=wq_sb, in_=wq_r)
            nc.sync.dma_start(out=wk_sb, in_=wk_r)
            nc.sync.dma_start(out=wo_sb, in_=wo_r)
            nc.sync.dma_start(out=cos_sb, in_=cost)
            nc.sync.dma_start(out=sin_sb, in_=sint)
            nc.sync.dma_start(out=btpl_sb, in_=btpl)
            nc.vector.memset(ones_f, 1.0)
            nc.vector.tensor_copy(ones, ones_f)

            # ---- A2 units: one s-chunk = Q half then K half (b0..b3) ---
            def a2_units(sc):
                ssl = slice(sc * 512, (sc + 1) * 512)
                units = []
                state = {}

                def prep():
                    state['xt'] = [xt_tile(sc, j) for j in range(NXT)]

                units.append(prep)
                for half, (w_sb, dts) in enumerate(
                        ((wq_sb, [qts[d][sc] for d in range(GH)]),
                         (wk_sb, [kts[d][sc] for d in range(GH)]))):
                    pss = {}

                    def step(j, hh, w_sb=w_sb, pss=pss):
                        h = j * XB + hh
                        if h == 0:
                            for d in range(GH):
                                pss[d] = bank(d)
                        for d in range(GH):
                            nc.tensor.matmul(
                                pss[d], w_sb[:, h, d * 128:(d + 1) * 128],
                                state['xt'][j][:, hh, :],
                                start=(h == 0), stop=(h == HC - 1))

                    for j in range(NXT):
                        for hh in range(XB):
                            units.append(lambda j=j, hh=hh, step=step: step(j, hh))

                    def evict(d, dsl, pss=pss):
                        t2 = work.tile([128, 512], F32, tag="t2", bufs=4,
                                       name=f"t2_{sc}_{d}_{dsl.tensor.name}")
                        nc.scalar.copy(t2, pss[d])       # frees the bank
                        t1 = work.tile([128, 512], F32, tag="t1", bufs=2,
                                       name=f"t1_{sc}_{d}_{dsl.tensor.name}")
                        nc.vector.tensor_mul(t1, t2, cos_sb[:, ssl])
                        nc.vector.tensor_mul(dsl[0:64, :], t2[64:128, :],
                                             sin_sb[64:128, ssl])
                        nc.vector.tensor_mul(dsl[64:128, :], t2[0:64, :],
                                             sin_sb[0:64, ssl])
                        nc.vector.tensor_add(dsl, dsl, t1)

                    for d in range(GH):
                        units.append(lambda d=d, dsl=dts[d], evict=evict:
                                     evict(d, dsl))
                return units

            # ---- B units: attention + o_proj for one q chunk -----------
            # banks: sps b4/b5, ctx b6, den b7, o_proj b4/b5
            def b_units(qc):
                if variant == "causal":
                    order = list(range(4 * qc, 4 * qc + 4)) + list(range(4 * qc))
                else:
                    order = list(range(HC))
                nkb = len(order)
                units = []
                ctx_t = []
                for hd in range(GH):
                    st = {}

                    def start_head(st=st):
                        st['ctxps'] = bank(6)
                        st['denps'] = bank(7, shape=(1, 512))
                        st['pend'] = None

                    def flush(last, st=st, hd=hd):
                        pexp, first, kbp = st['pend']
                        nc.tensor.matmul(st['ctxps'],
                                         vts[kbp][:, hd * 128:(hd + 1) * 128],
                                         pexp, start=first, stop=last)
                        nc.tensor.matmul(st['denps'], ones, pexp,
                                         start=first, stop=last)

                    def kb_iter(i, kb, st=st, hd=hd, start_head=start_head, flush=flush):
                        if i == 0:
                            start_head()
                        sps = bank(4 + i % 2)
                        nc.tensor.matmul(
                            sps,
                            kts[hd][kb // 4][:, (kb % 4) * 128:(kb % 4 + 1) * 128],
                            qts[hd][qc], start=True, stop=True)
                        if variant == "causal" and kb >= 4 * qc:
                            j = kb - 4 * qc
                            off = (3 - j) * 128
                            w = (j + 1) * 128
                            nc.vector.tensor_add(sps[:, 0:w], sps[:, 0:w],
                                                 btpl_sb[:, off:off + w])
                        pexp = work.tile([128, 512], MDT, tag="pexp", bufs=3,
                                         name=f"pexp_{qc}_{hd}_{kb}")
                        nc.scalar.activation(pexp, sps, EXP, scale=SCALE)
                        if st['pend'] is not None:
                            flush(False)
                        st['pend'] = (pexp, i == 0, kb)

                    def tail(st=st, hd=hd, flush=flush):
                        flush(True)
                        dsm = work.tile([1, 512], F32, tag="dsm", bufs=2,
                                        name=f"dsm_{qc}_{hd}")
                        nc.scalar.copy(dsm, st['denps'])
                        dbc = work.tile([128, 512], F32, tag="dbc", bufs=1,
                                        name=f"dbc_{qc}_{hd}")
                        nc.gpsimd.partition_broadcast(dbc, dsm)
                        nc.vector.reciprocal(dbc, dbc)
                        ct = work.tile([128, 512], MDT, tag="ctx", bufs=6,
                                       name=f"ctx_{qc}_{hd}")
                        nc.vector.tensor_mul(ct, st['ctxps'], dbc)
                        ctx_t.append(ct)

                    for i, kb in enumerate(order):
                        units.append(lambda i=i, kb=kb, kb_iter=kb_iter:
                                     kb_iter(i, kb))
                    units.append(tail)

                for qb in range(4):
                    st2 = {}

                    def oproj(qb, ob, st2=st2):
                        if ob == 0:
                            st2['orow'] = work.tile([128, HID], F32, tag="orow",
                                                    bufs=2, name=f"orow_{qc}_{qb}")
                        ops = bank(4 + ob % 2)
                        for hd in range(GH):
                            nc.tensor.matmul(
                                ops, ctx_t[hd][:, qb * 128:(qb + 1) * 128],
                                wo_sb[:, hd, ob * 512:(ob + 1) * 512],
                                start=(hd == 0), stop=(hd == GH - 1))
                        nc.vector.tensor_copy(
                            st2['orow'][:, ob * 512:(ob + 1) * 512], ops)
                        if ob == 3:
                            nc.sync.dma_start(
                                out=out[(qc * 4 + qb) * 128:
                                        (qc * 4 + qb + 1) * 128, :],
                                in_=st2['orow'])

                    for ob in range(4):
                        units.append(lambda qb=qb, ob=ob, oproj=oproj:
                                     oproj(qb, ob))
                return units

            # ---- emit: A2-sc0 alone, then interleave B-qc & A2-sc(qc+1)
            for u in a2_units(0):
                u()
            for qc in range(SC):
                bu = b_units(qc)
                au = a2_units(qc + 1) if qc + 1 < SC else []
                na, nb = len(au), len(bu)
                ai = 0
                for i, u in enumerate(bu):
                    u()
                    tgt = (i + 1) * na // nb
                    while ai < tgt:
                        au[ai]()
                        ai += 1
                while ai < na:
                    au[ai]()
                    ai += 1
    nc.compile()
    return nc


_CACHE = {}


def _get(variant, dt=None):
    dt = dt or DTYPE
    if (variant, dt) not in _CACHE:
        _CACHE[(variant, dt)] = _build(variant, dt)
    return _CACHE[(variant, dt)]


def _rope_tables():
    inv = 1.0 / (10000.0 ** (np.arange(0, D, 2, dtype=np.float64) / D))  # [64]
    t = np.arange(S, dtype=np.float64)
    fr = np.outer(inv, t)                       # [64, S]
    cosT = np.concatenate([np.cos(fr), np.cos(fr)], 0).astype(np.float32)
    # partition-swapped sign-folded sin: rows 0:64 = +sin, rows 64:128 = -sin
    sinT = np.concatenate([np.sin(fr), -np.sin(fr)], 0).astype(np.float32)
    return cosT, sinT


def _btpl_causal():
    # additive mask template: NEG where k > c-384 else 0
    k = np.arange(128)[:, None]
    c = np.arange(896)[None, :]
    return np.where(k > c - 384, np.float32(NEG), np.float32(0.0)).astype(np.float32)


def _np_cast(a, dt):
    if dt == "f16":
        return a.astype(np.float16)
    if dt == "bf16":
        import ml_dtypes
        return a.astype(ml_dtypes.bfloat16)
    return a


def _numpy_fallback(hs, Wq, Wk, Wv, Wo, mask):
    B = hs.shape[0]
    cosT, sinT = _rope_tables()
    cos = cosT.T[None, :, None, :]
    sin = np.abs(sinT).T[None, :, None, :]
    outs = []
    for b in range(B):
        x = hs[b]
        q = (x @ Wq).reshape(S, 16, D)[None]
        k = (x @ Wk).reshape(S, 16, D)[None]
        vv = (x @ Wv).reshape(S, 16, D)

        def rope(z):
            z1, z2 = z[..., :64], z[..., 64:]
            rot = np.concatenate([-z2, z1], -1)
            return z * cos + rot * sin

        q, k = rope(q)[0], rope(k)[0]
        o = np.empty((S, 16, D), np.float32)
        m = mask[0, 0]
        for h in range(16):
            sc = (q[:, h] @ k[:, h].T) * SCALE
            sc = np.where(m == 0, -np.inf, sc)
            sc -= sc.max(-1, keepdims=True)
            p = np.exp(sc)
            p /= p.sum(-1, keepdims=True)
            o[:, h] = p @ vv[:, h]
        outs.append(o.reshape(S, HID) @ Wo)
    return np.stack(outs).astype(np.float32)


def kernel(hidden_states, Wq, Wk, Wv, Wo, attention_mask):
    hs = np.asarray(hidden_states, dtype=np.float32)
    Wq, Wk, Wv, Wo = (np.asarray(w, dtype=np.float32) for w in (Wq, Wk, Wv, Wo))
    mask = np.asarray(attention_mask)
    B = hs.shape[0]

    m2 = mask.reshape(mask.shape[-2], mask.shape[-1])
    if np.all(m2 == 1):
        variant = "full"
    elif np.array_equal(m2 != 0, np.tril(np.ones((S, S), dtype=bool))):
        variant = "causal"
    else:
        return _numpy_fallback(hs, Wq, Wk, Wv, Wo, mask)

    cosT, sinT = _rope_tables()
    btpl = _btpl_causal() if variant == "causal" else np.zeros((128, 896), np.float32)

    in_maps = []
    for c in range(NCORES):
        b, g = divmod(c, GH)
        gsl = slice(g * GW, (g + 1) * GW)
        in_maps.append({
            "xt": _np_cast(np.ascontiguousarray(hs[b].T), DTYPE),
            "wq": _np_cast(np.ascontiguousarray(Wq[:, gsl]), DTYPE),
            "wk": _np_cast(np.ascontiguousarray(Wk[:, gsl]), DTYPE),
            "wv": _np_cast(np.ascontiguousarray(Wv[:, gsl]), DTYPE),
            "wo": _np_cast(np.ascontiguousarray(Wo[gsl, :]), DTYPE),
            "cost": cosT, "sint": sinT, "btpl": btpl,
        })

    nc = _get(variant)
    res = run_bass_kernel_spmd(nc, in_maps, list(range(NCORES))).results
    out = np.zeros((B, S, HID), np.float32)
    for c in range(NCORES):
        b = c // GH
        out[b] += res[c]["out"]
    return out


# revision 16
# speedup vs baseline: 1.3601x; 1.0037x over previous
"""Trainium2 Bass kernel for nn_Attention (dense transformer attention block).

Full-input contract: kernel(**inputs) takes the unsharded inputs and returns
the full output. 8 NeuronCores: tensor-parallel over head groups (4 heads) x
data-parallel over batch (2); core c = b*4 + g. Per core: q/k/v projections
for its head group, RoPE, causal flash-style attention (transposed-P layout,
softmax without max-subtraction), partial o_proj with its rows of Wo; the 4
partials per batch element are summed on the host (the all-reduce of the
row-sharded o_proj).

The Q/K projection work (phase A2) is interleaved into the attention stream
(phase B) as PE filler: B-qc only needs Q/K columns of chunks <= qc, so
A2-sc(qc+1) runs alongside B-qc, hiding the exp/softmax latency chains.

Matmul dtype configurable (DTYPE): fp16 default (~7e-4 rel err), f32r
fallback (~3.5e-4).
"""
import contextlib
import numpy as np
import concourse.bass as bass
from concourse import bacc
import concourse.mybir as mybir
import concourse.tile as tile
from concourse.bass_utils import run_bass_kernel_spmd

F32 = mybir.dt.float32
F32R = mybir.dt.float32r
F16 = mybir.dt.float16
BF16 = mybir.dt.bfloat16
EXP = mybir.ActivationFunctionType.Exp
MMDT = {"f32r": F32R, "f16": F16, "bf16": BF16}

S = 2048
HID = 2048
D = 128
GH = 4            # heads per core
GW = GH * D       # 512
NCORES = 8
SC = S // 512     # 4 column chunks
HC = HID // 128   # 16 contraction chunks
SCALE = float(D) ** -0.5
NEG = -1.0e30

DTYPE = "f16"     # matmul dtype: 'f16' | 'bf16' | 'f32r'


def _build(variant, dt):
    MDT = MMDT[dt]
    two_byte = dt in ("f16", "bf16")
    IDT = MDT if two_byte else F32
    nc = bacc.Bacc("TRN2", target_bir_lowering=False, debug=False,
                   num_devices=NCORES)
    xt = nc.dram_tensor("xt", [HID, S], IDT, kind="ExternalInput").ap()
    wq = nc.dram_tensor("wq", [HID, GW], IDT, kind="ExternalInput").ap()
    wk = nc.dram_tensor("wk", [HID, GW], IDT, kind="ExternalInput").ap()
    wv = nc.dram_tensor("wv", [HID, GW], IDT, kind="ExternalInput").ap()
    wo = nc.dram_tensor("wo", [GW, HID], IDT, kind="ExternalInput").ap()
    cost = nc.dram_tensor("cost", [D, S], F32, kind="ExternalInput").ap()
    sint = nc.dram_tensor("sint", [D, S], F32, kind="ExternalInput").ap()
    btpl = nc.dram_tensor("btpl", [D, 896], F32, kind="ExternalInput").ap()
    out = nc.dram_tensor("out", [S, HID], F32, kind="ExternalOutput").ap()

    def _bc(ap):
        return ap if two_byte else ap.bitcast(F32R)

    xt_r = _bc(xt.rearrange("(c p) s -> p c s", p=128))   # [128, 16, 2048]
    wq_r = _bc(wq.rearrange("(c p) m -> p c m", p=128))   # [128, 16, 512]
    wk_r = _bc(wk.rearrange("(c p) m -> p c m", p=128))
    wv_r = _bc(wv.rearrange("(c p) m -> p c m", p=128))
    wo_r = _bc(wo.rearrange("(c p) m -> p c m", p=128))   # [128, 4, 2048]

    XB = 4                   # h-chunks per xt DMA
    NXT = HC // XB           # 4 xt tiles per s-chunk

    with tile.TileContext(nc) as tc:
        with contextlib.ExitStack() as ctx:
            persist = ctx.enter_context(tc.tile_pool(name="persist", bufs=1))
            psum = ctx.enter_context(tc.tile_pool(name="psum", bufs=1, space="PSUM"))
            work = ctx.enter_context(tc.tile_pool(name="work", bufs=1))

            _n = [0]

            def bank(i, shape=(128, 512)):
                _n[0] += 1
                return psum.tile(list(shape), F32, tag=f"b{i}", name=f"bk{i}_{_n[0]}")

            qts = [[persist.tile([128, 512], MDT, tag=f"qt{h}_{s}",
                                 name=f"qt{h}_{s}") for s in range(SC)]
                   for h in range(GH)]
            kts = [[persist.tile([128, 512], MDT, tag=f"kt{h}_{s}",
                                 name=f"kt{h}_{s}") for s in range(SC)]
                   for h in range(GH)]
            vts = [persist.tile([128, GW], MDT, tag=f"v{st}", name=f"v{st}")
                   for st in range(HC)]
            cos_sb = persist.tile([128, S], F32, tag="cos")
            sin_sb = persist.tile([128, S], F32, tag="sin")
            btpl_sb = persist.tile([128, 896], F32, tag="btpl")
            ones_f = persist.tile([128, 1], F32, tag="onesf")
            ones = persist.tile([128, 1], MDT, tag="ones")
            wo_sb = persist.tile([128, GH, HID], MDT, tag="wo")

            def xt_tile(sc, j):
                t = work.tile([128, XB, 512], MDT, tag="xt", bufs=6,
                              name=f"xt_{sc}_{j}")
                nc.sync.dma_start(
                    out=t, in_=xt_r[:, j * XB:(j + 1) * XB,
                                    sc * 512:(sc + 1) * 512])
                return t

            # ---- A1: V = X @ Wv (banks b0..b3) -------------------------
            with tc.tile_pool(name="phV", bufs=1) as phv:
                first_xt = xt_tile(0, 0)
                wv_cs = []
                for j in range(NXT):
                    wvc = phv.tile([128, XB, GW], MDT, tag="wvf", bufs=NXT,
                                   name=f"wvf_{j}")
                    nc.sync.dma_start(out=wvc, in_=wv_r[:, j * XB:(j + 1) * XB, :])
                    wv_cs.append(wvc)
                for sc in range(SC):
                    vps = [bank(st) for st in range(4)]
                    for j in range(NXT):
                        xt_t = first_xt if (sc == 0 and j == 0) else xt_tile(sc, j)
                        for hh in range(XB):
                            h = j * XB + hh
                            for st in range(4):
                                nc.tensor.matmul(
                                    vps[st], xt_t[:, hh, st * 128:(st + 1) * 128],
                                    wv_cs[j][:, hh, :],
                                    start=(h == 0), stop=(h == HC - 1))
                    for st in range(4):
                        nc.scalar.copy(vts[sc * 4 + st], vps[st])

            # weights for Q/K/O + tables, loaded during A1
            wq_sb = work.tile([128, HC, GW], MDT, tag="wqk", bufs=2)
            wk_sb = work.tile([128, HC, GW], MDT, tag="wqk", bufs=2)
            nc.sync.dma_start(out=wq_sb, in_=wq_r)
            nc.sync.dma_start(out=wk_sb, in_=wk_r)
            nc.sync.dma_start(out=wo_sb, in_=wo_r)
            nc.sync.dma_start(out=cos_sb, in_=cost)
            nc.sync.dma_start(out=sin_sb, in_=sint)
            nc.sync.dma_start(out=btpl_sb, in_=btpl)
            nc.vector.memset(ones_f, 1.0)
            nc.vector.tensor_copy(ones, ones_f)

            # ---- A2 units: one s-chunk = Q half then K half (b0..b3) ---
            def a2_units(sc):
                ssl = slice(sc * 512, (sc + 1) * 512)
                units = []
                state = {}

                def prep():
                    state['xt'] = [xt_tile(sc, j) for j in range(NXT)]

                units.append(prep)
                for half, (w_sb, dts) in enumerate(
                        ((wq_sb, [qts[d][sc] for d in range(GH)]),
                         (wk_sb, [kts[d][sc] for d in range(GH)]))):
                    pss = {}

                    def step(j, hh, w_sb=w_sb, pss=pss):
                        h = j * XB + hh
                        if h == 0:
                            for d in range(GH):
                                pss[d] = bank(d)
                        for d in range(GH):
                            nc.tensor.matmul(
                                pss[d], w_sb[:, h, d * 128:(d + 1) * 128],
                                state['xt'][j][:, hh, :],
                                start=(h == 0), stop=(h == HC - 1))

                    for j in range(NXT):
                        for hh in range(XB):
                            units.append(lambda j=j, hh=hh, step=step: step(j, hh))

                    def evict(d, dsl, pss=pss):
                        t2 = work.tile([128, 512], F32, tag="t2", bufs=4,
                                       name=f"t2_{sc}_{d}_{dsl.tensor.name}")
                        nc.scalar.copy(t2, pss[d])       # frees the bank
                        t1 = work.tile([128, 512], F32, tag="t1", bufs=2,
                                       name=f"t1_{sc}_{d}_{dsl.tensor.name}")
                        nc.vector.tensor_mul(t1, t2, cos_sb[:, ssl])
                        nc.vector.tensor_mul(dsl[0:64, :], t2[64:128, :],
                                             sin_sb[64:128, ssl])
                        nc.vector.tensor_mul(dsl[64:128, :], t2[0:64, :],
                                             sin_sb[0:64, ssl])
                        nc.vector.tensor_add(dsl, dsl, t1)

                    for d in range(GH):
                        units.append(lambda d=d, dsl=dts[d], evict=evict:
                                     evict(d, dsl))
                return units

            # ---- B units: attention + o_proj for one q chunk -----------
            # banks: sps b4/b5, ctx b6, den b7, o_proj b4/b5
            def b_units(qc):
                if variant == "causal":
                    order = list(range(4 * qc, 4 * qc + 4)) + list(range(4 * qc))
                else:
                    order = list(range(HC))
                nkb = len(order)
                units = []
                ctx_t = []
                for hd in range(GH):
                    st = {}

                    def start_head(st=st, hd=hd):
                        if qc == SC - 1:   # no A2 filler; b0..b3 are free
                            st['ctxps'] = bank(hd % 2)
                            st['denps'] = bank(2 + hd % 2, shape=(1, 512))
                        else:
                            st['ctxps'] = bank(6)
                            st['denps'] = bank(7, shape=(1, 512))
                        st['pend'] = None

                    def flush(last, st=st, hd=hd):
                        pexp, first, kbp = st['pend']
                        nc.tensor.matmul(st['ctxps'],
                                         vts[kbp][:, hd * 128:(hd + 1) * 128],
                                         pexp, start=first, stop=last)
                        nc.tensor.matmul(st['denps'], ones, pexp,
                                         start=first, stop=last)

                    def kb_iter(i, kb, st=st, hd=hd, start_head=start_head, flush=flush):
                        if i == 0:
                            start_head()
                        sps = bank(4 + i % 2)
                        nc.tensor.matmul(
                            sps,
                            kts[hd][kb // 4][:, (kb % 4) * 128:(kb % 4 + 1) * 128],
                            qts[hd][qc], start=True, stop=True)
                        if variant == "causal" and kb >= 4 * qc:
                            j = kb - 4 * qc
                            off = (3 - j) * 128
                            w = (j + 1) * 128
                            nc.vector.tensor_add(sps[:, 0:w], sps[:, 0:w],
                                                 btpl_sb[:, off:off + w])
                        pexp = work.tile([128, 512], MDT, tag="pexp", bufs=3,
                                         name=f"pexp_{qc}_{hd}_{kb}")
                        nc.scalar.activation(pexp, sps, EXP, scale=SCALE)
                        if st['pend'] is not None:
                            flush(False)
                        st['pend'] = (pexp, i == 0, kb)

                    def tail(st=st, hd=hd, flush=flush):
                        flush(True)
                        dsm = work.tile([1, 512], F32, tag="dsm", bufs=2,
                                        name=f"dsm_{qc}_{hd}")
                        nc.scalar.copy(dsm, st['denps'])
                        dbc = work.tile([128, 512], F32, tag="dbc", bufs=1,
                                        name=f"dbc_{qc}_{hd}")
                        nc.gpsimd.partition_broadcast(dbc, dsm)
                        nc.vector.reciprocal(dbc, dbc)
                        ct = work.tile([128, 512], MDT, tag="ctx", bufs=6,
                                       name=f"ctx_{qc}_{hd}")
                        nc.vector.tensor_mul(ct, st['ctxps'], dbc)
                        ctx_t.append(ct)

                    for i, kb in enumerate(order):
                        units.append(lambda i=i, kb=kb, kb_iter=kb_iter:
                                     kb_iter(i, kb))
                    units.append(tail)

                for qb in range(4):
                    st2 = {}

                    def oproj(qb, ob, st2=st2):
                        ops = bank(6 + ob % 2)
                        for hd in range(GH):
                            nc.tensor.matmul(
                                ops, ctx_t[hd][:, qb * 128:(qb + 1) * 128],
                                wo_sb[:, hd, ob * 512:(ob + 1) * 512],
                                start=(hd == 0), stop=(hd == GH - 1))
                        ot = work.tile([128, 512], F32, tag="outsb", bufs=4,
                                       name=f"ot_{qc}_{qb}_{ob}")
                        if ob % 2 == 0:
                            nc.scalar.copy(ot, ops)
                        else:
                            nc.vector.tensor_copy(ot, ops)
                        nc.sync.dma_start(
                            out=out[(qc * 4 + qb) * 128:(qc * 4 + qb + 1) * 128,
                                    ob * 512:(ob + 1) * 512],
                            in_=ot)

                    for ob in range(4):
                        units.append(lambda qb=qb, ob=ob, oproj=oproj:
                                     oproj(qb, ob))
                return units

            # ---- emit: A2-sc0 alone, then interleave B-qc & A2-sc(qc+1)
            for u in a2_units(0):
                u()
            for qc in range(SC):
                bu = b_units(qc)
                au = a2_units(qc + 1) if qc + 1 < SC else []
                na, nb = len(au), len(bu)
                ai = 0
                for i, u in enumerate(bu):
                    u()
                    tgt = (i + 1) * na // nb
                    while ai < tgt:
                        au[ai]()
                        ai += 1
                while ai < na:
                    au[ai]()
                    ai += 1
    nc.compile()
    return nc


_CACHE = {}


def _get(variant, dt=None):
    dt = dt or DTYPE
    if (variant, dt) not in _CACHE:
        _CACHE[(variant, dt)] = _build(variant, dt)
    return _CACHE[(variant, dt)]


def _rope_tables():
    inv = 1.0 / (10000.0 ** (np.arange(0, D, 2, dtype=np.float64) / D))  # [64]
    t = np.arange(S, dtype=np.float64)
    fr = np.outer(inv, t)                       # [64, S]
    cosT = np.concatenate([np.cos(fr), np.cos(fr)], 0).astype(np.float32)
    # partition-swapped sign-folded sin: rows 0:64 = +sin, rows 64:128 = -sin
    sinT = np.concatenate([np.sin(fr), -np.sin(fr)], 0).astype(np.float32)
    return cosT, sinT


def _btpl_causal():
    # additive mask template: NEG where k > c-384 else 0
    k = np.arange(128)[:, None]
    c = np.arange(896)[None, :]
    return np.where(k > c - 384, np.float32(NEG), np.float32(0.0)).astype(np.float32)


def _np_cast(a, dt):
    if dt == "f16":
        return a.astype(np.float16)
    if dt == "bf16":
        import ml_dtypes
        return a.astype(ml_dtypes.bfloat16)
    return a


def _numpy_fallback(hs, Wq, Wk, Wv, Wo, mask):
    B = hs.shape[0]
    cosT, sinT = _rope_tables()
    cos = cosT.T[None, :, None, :]
    sin = np.abs(sinT).T[None, :, None, :]
    outs = []
    for b in range(B):
        x = hs[b]
        q = (x @ Wq).reshape(S, 16, D)[None]
        k = (x @ Wk).reshape(S, 16, D)[None]
        vv = (x @ Wv).reshape(S, 16, D)

        def rope(z):
            z1, z2 = z[..., :64], z[..., 64:]
            rot = np.concatenate([-z2, z1], -1)
            return z * cos + rot * sin

        q, k = rope(q)[0], rope(k)[0]
        o = np.empty((S, 16, D), np.float32)
        m = mask[0, 0]
        for h in range(16):
            sc = (q[:, h] @ k[:, h].T) * SCALE
            sc = np.where(m == 0, -np.inf, sc)
            sc -= sc.max(-1, keepdims=True)
            p = np.exp(sc)
            p /= p.sum(-1, keepdims=True)
            o[:, h] = p @ vv[:, h]
        outs.append(o.reshape(S, HID) @ Wo)
    return np.stack(outs).astype(np.float32)


def kernel(hidden_states, Wq, Wk, Wv, Wo, attention_mask):
    hs = np.asarray(hidden_states, dtype=np.float32)
    Wq, Wk, Wv, Wo = (np.asarray(w, dtype=np.float32) for w in (Wq, Wk, Wv, Wo))
    mask = np.asarray(attention_mask)
    B = hs.shape[0]

    m2 = mask.reshape(mask.shape[-2], mask.shape[-1])
    if np.all(m2 == 1):
        variant = "full"
    elif np.array_equal(m2 != 0, np.tril(np.ones((S, S), dtype=bool))):
        variant = "causal"
    else:
        return _numpy_fallback(hs, Wq, Wk, Wv, Wo, mask)

    cosT, sinT = _rope_tables()
    btpl = _btpl_causal() if variant == "causal" else np.zeros((128, 896), np.float32)

    in_maps = []
    for c in range(NCORES):
        b, g = divmod(c, GH)
        gsl = slice(g * GW, (g + 1) * GW)
        in_maps.append({
            "xt": _np_cast(np.ascontiguousarray(hs[b].T), DTYPE),
            "wq": _np_cast(np.ascontiguousarray(Wq[:, gsl]), DTYPE),
            "wk": _np_cast(np.ascontiguousarray(Wk[:, gsl]), DTYPE),
            "wv": _np_cast(np.ascontiguousarray(Wv[:, gsl]), DTYPE),
            "wo": _np_cast(np.ascontiguousarray(Wo[gsl, :]), DTYPE),
            "cost": cosT, "sint": sinT, "btpl": btpl,
        })

    nc = _get(variant)
    res = run_bass_kernel_spmd(nc, in_maps, list(range(NCORES))).results
    out = np.zeros((B, S, HID), np.float32)
    for c in range(NCORES):
        b = c // GH
        out[b] += res[c]["out"]
    return out
